# revision 35
# baseline (speedup 1.0000x reference)
"""Trainium2 Bass kernel for CRFExtensionModule (conv3x3 backbone + 5 mean-field
CRF iterations with separable Gaussian blur).

Strategy (per NeuronCore, 2 images A/B of the 16-image batch):
  - C=2 softmax collapses to a sigmoid of d = logit1 - logit0, and
    blur(q0) = blur(ones) - blur(q1), so the whole CRF loop is a single-plane
    recurrence:  d' = (du - ob) + 2*blur(sigmoid(d)),  ob = blur(ones).
  - conv3x3 computes only the planes u1 and du = u1 - u0 via banded matmuls on
    the TensorEngine (band stationary, x moving), with tiny K=35 "fix" matmuls
    for the 2 rows per 128-row tile boundary that the aligned K window misses.
  - blur = two *transposing* banded matmul passes on the TensorEngine:
    pass1: UT[w,h'] = sum_h s[h,w] A[h,h']   (lhsT = s block, rhs = A band)
    pass2: V[h,w'] = sum_w UT[w,h] A[w,w']   (lhsT = UT block, rhs = A band)
  - M = ob - du is injected into the pass-2 PSUM with K=128 negI matmuls, so
    sigmoid reads PSUM directly (d' = 2*blur(s) - M lives in the bank).
  - Scheduling: the two images' CRF iterations are interleaved on the PE
    (A0 | convB | A1 | B0 | A2 | B1 | ...), so image X's sigmoid chain runs
    under image Y's matmuls and the PE almost never stalls.
  - Consts stream on the (otherwise idle) Sync HWDGE ring; x streams on the
    SWDGE cast ring in (channel-interleaved, row-tile) order so the first conv
    matmul can start ~5us in.  Outputs are written fp16 (tolerance is 2e-2;
    fp16 adds ~5e-4) and all output DMAs issue from the Sync ring.
"""

import os
import sys
from contextlib import ExitStack

sys.path.insert(0, "/opt/trn_rl_repo")

import numpy as np
import ml_dtypes

import concourse.bass as bass
import concourse.bacc as bacc
import concourse.tile as tile
import concourse.mybir as mybir
from concourse.bass_utils import run_bass_kernel_spmd

F32 = mybir.dt.float32
BF16 = mybir.dt.bfloat16
FP16 = mybir.dt.float16

N_CORES = 8
IMGS_PER_CORE = 2
H = W = 512
NT = 4  # 128-row tiles per image plane
N_ITER = 5
FILT = 11


def _gauss_k():
    d = np.arange(FILT, dtype=np.float32) - np.float32((FILT - 1) / 2.0)
    k = np.exp(-(d ** 2) / np.float32(2.0)).astype(np.float32)
    return (k / k.sum()).astype(np.float32)


def _make_A(scale):
    """A[h, h'] = k[h-h'+5] for |h-h'| <= 5 (zero-padded 'SAME' 1D blur)."""
    k = (_gauss_k() * np.float32(scale)).astype(np.float32)
    A = np.zeros((H, H), np.float32)
    hp = np.arange(H)
    for j in range(FILT):
        h = hp + (j - 5)
        m = (h >= 0) & (h < H)
        A[h[m], hp[m]] = k[j]
    return A


def _win(t):
    """h' window that rows [128t, 128t+128) of A touch."""
    return max(0, 128 * t - 5), min(H, 128 * t + 133)


def _np_dt(dt):
    if dt == BF16:
        return ml_dtypes.bfloat16
    if dt == FP16:
        return np.float16
    return np.float32


# ---------------------------------------------------------------------------
# kernel body (traced once; shared SPMD program for all 8 cores)
# ---------------------------------------------------------------------------


def _build(nc, tc, conv_dt, blur_dt):
    x_d = nc.dram_tensor("x", [IMGS_PER_CORE, 3, H, W], FP16, kind="ExternalInput").ap()
    y_d = nc.dram_tensor("y", [IMGS_PER_CORE, 2, H, W], FP16, kind="ExternalOutput").ap()
    bands_d = nc.dram_tensor("bands", [128, 18, 128], conv_dt, kind="ExternalInput").ap()
    wf_d = nc.dram_tensor("wf", [35, 6, 128], conv_dt, kind="ExternalInput").ap()
    As_d = nc.dram_tensor("A_s", [128, NT, H], blur_dt, kind="ExternalInput").ap()
    Ap_d = nc.dram_tensor("A_p", [128, NT, H], blur_dt, kind="ExternalInput").ap()
    negI_d = nc.dram_tensor("negI", [128, 128], FP16, kind="ExternalInput").ap()
    ob_d = nc.dram_tensor("ob", [128, NT, W], FP16, kind="ExternalInput").ap()
    biases_d = nc.dram_tensor("biases", [128, 2], F32, kind="ExternalInput").ap()

    ALU = mybir.AluOpType
    AF = mybir.ActivationFunctionType

    with ExitStack() as ctx:
        cpool = ctx.enter_context(tc.tile_pool(name="consts", bufs=1))
        spool = ctx.enter_context(tc.tile_pool(name="sbuf", bufs=1))
        xpool = ctx.enter_context(tc.tile_pool(name="xin", bufs=1))
        ppool = ctx.enter_context(
            tc.tile_pool(name="psum", bufs=8, space=bass.MemorySpace.PSUM))

        def psum():
            return ppool.tile([128, 512], F32, tag="ps", name="ps")

        # --- constants into SBUF on the Sync HWDGE ring, in order of first
        #     need (Scalar's ring is blocked by ACT table loads; Sync is
        #     otherwise idle).  x goes on the SWDGE ring. ---
        bands = cpool.tile([128, 18, 128], conv_dt, tag="bands", name="bands")
        nc.sync.dma_start(bands[:], bands_d)
        biases = cpool.tile([128, 2], F32, tag="biases", name="biases")
        nc.sync.dma_start(biases[:], biases_d)
        wf = cpool.tile([35, 6, 128], conv_dt, tag="wf", name="wf")
        nc.sync.dma_start(wf[:], wf_d)
        A_s = cpool.tile([128, NT, H], blur_dt, tag="A_s", name="A_s")
        nc.sync.dma_start(A_s[:], As_d)
        ob = cpool.tile([128, NT, W], FP16, tag="ob", name="ob")
        nc.sync.dma_start(ob[:], ob_d)
        negI = cpool.tile([128, 128], FP16, tag="negI", name="negI")
        nc.sync.dma_start(negI[:], negI_d)
        A_p = cpool.tile([128, NT, H], blur_dt, tag="A_p", name="A_p")
        nc.sync.dma_start(A_p[:], Ap_d)

        # --- per-image state ---
        xt, xbt, u1, du, M, P2, s_sb, ut, dp = {}, {}, {}, {}, {}, {}, {}, {}, {}
        for im in range(IMGS_PER_CORE):
            xt[im] = xpool.tile([128, 3, NT, W], conv_dt, tag=f"xt{im}", name=f"xt{im}")
            xbt[im] = [xpool.tile([35, W], conv_dt, tag=f"xb{im}{b}", name=f"xb{im}{b}")
                       for b in range(NT)]
            u1[im] = [spool.tile([128, W], F32, tag=f"u1_{im}_{b}", name=f"u1_{im}_{b}") for b in range(NT)]
            du[im] = [spool.tile([128, W], F32, tag=f"du_{im}_{b}", name=f"du_{im}_{b}") for b in range(NT)]
            M[im] = [spool.tile([128, W], FP16, tag=f"M_{im}_{b}", name=f"M_{im}_{b}") for b in range(NT)]
            P2[im] = [spool.tile([128, W], F32, tag=f"P2_{im}_{b}", name=f"P2_{im}_{b}") for b in range(NT)]
            dp[im] = None

        def s_tiles(im):
            return [spool.tile([128, W], blur_dt, tag=f"s_{im}_{t}", name=f"s_{im}_{t}") for t in range(NT)]

        def ut_tiles(im):
            return [spool.tile([128, H], blur_dt, tag=f"ut_{im}_{s}", name=f"ut_{im}_{s}") for s in range(NT)]

        # --- x DMAs: memsets first (vector), then fp16 loads in (tile,
        #     channel) order so conv tile b is fed just in time ---
        for im in range(IMGS_PER_CORE):
            for b in range(NT):
                nc.vector.memset(xbt[im][b][:], 0.0)

        def x_tile(im, b, ring):
            for c in range(3):
                ring.dma_start(
                    xt[im][:, c, b, :], x_d[im, c, 128 * b:128 * b + 128, :])
            if b > 0:
                ring.dma_start(xbt[im][b][0:3, :], x_d[im, :, 128 * b - 1, :])
            if b < NT - 1:
                ring.dma_start(xbt[im][b][32:35, :], x_d[im, :, 128 * b + 128, :])

        for b in range(NT):
            x_tile(0, b, nc.gpsimd)
        # image B: per-channel whole-plane issues (fewer descriptors on the
        # SWDGE queue); its boundary strips ride the Sync ring
        for c in range(3):
            nc.gpsimd.dma_start(
                xt[1][:, c, :, :],
                x_d[1, c].rearrange("(b p) w -> p b w", p=128))
        for b in range(NT):
            if b > 0:
                nc.sync.dma_start(xbt[1][b][0:3, :], x_d[1, :, 128 * b - 1, :])
            if b < NT - 1:
                nc.sync.dma_start(xbt[1][b][32:35, :], x_d[1, :, 128 * b + 128, :])

        # --- emission helpers (program order == per-engine issue order) ---

        def conv_chunk(im, b, set_i):
            P = psum()
            n_mm = 0
            for c in range(3):
                for kx in (1, 0, 2):  # center first: full-bank start=True
                    sl, ol = (0, 1) if kx == 0 else (1, 0) if kx == 2 else (0, 0)
                    n = W - (1 if kx != 1 else 0)
                    nc.tensor.matmul(
                        P[:, ol:ol + n],
                        bands[:, set_i * 9 + c * 3 + kx, :],
                        xt[im][:, c, b, sl:sl + n],
                        start=(n_mm == 0), stop=False, skip_group_check=True)
                    n_mm += 1
            for kx in (1, 0, 2):
                sl, ol = (0, 1) if kx == 0 else (1, 0) if kx == 2 else (0, 0)
                n = W - (1 if kx != 1 else 0)
                nc.tensor.matmul(
                    P[:, ol:ol + n], wf[:, set_i * 3 + kx, :],
                    xbt[im][b][:, sl:sl + n],
                    start=False, stop=(kx == 2), skip_group_check=True)
            if set_i == 0:
                # u1 = conv1 + b1   (ACT)
                nc.scalar.activation(
                    u1[im][b][:], P[:], AF.Identity, bias=biases[:, 0:1], scale=1.0)
            else:
                # du = (conv1-conv0) + (b1-b0)   (DVE)
                nc.vector.tensor_scalar(
                    du[im][b][:], P[:], biases[:, 1:2], None, ALU.add)
                # M = ob - du (fp16) ; P2 = 2*u1 + M  (Pool: SBUF-only
                # operands; with this the final out0 = P2 - out1 is SBUF-only
                # too and runs on Pool, halving the DVE tail chain)
                nc.gpsimd.tensor_sub(M[im][b][:], ob[:, b, :], du[im][b][:])
                nc.gpsimd.tensor_add(P2[im][b][:], u1[im][b][:], M[im][b][:])
                nc.gpsimd.tensor_add(P2[im][b][:], P2[im][b][:], u1[im][b][:])

        def emit_sig(im, it):
            """s = sigmoid(d) for the next iteration's pass 1 (ACT)."""
            s_sb = s_tiles(im)
            src = du[im] if it == 0 else dp[im]
            for t in range(NT):
                nc.scalar.activation(s_sb[t][:], src[t][:], AF.Sigmoid)
            return s_sb

        sig = {}  # im -> current s tiles

        def emit_p1(im, it):
            A_t = A_s if it < N_ITER - 1 else A_p
            s_sb = sig[im]
            # pass 1 (bank-sequential: bank s completes at (s+1)/4 of the pass
            # so its SBUF copy overlaps the rest of the pass)
            uts = ut_tiles(im)
            for s in range(NT):
                P = psum()
                for t in range(NT):
                    lo, hi = _win(t)
                    nc.tensor.matmul(
                        P[:, lo:hi], s_sb[t][:, 128 * s:128 * s + 128],
                        A_t[:, t, lo:hi],
                        start=(t == 0), stop=(t == NT - 1), skip_group_check=True)
                # copy engines: s=0 first-needed by pass 2, s=3 last
                # (Pool cannot touch PSUM, so DVE takes 3 of 4)
                if s == 1:
                    nc.scalar.copy(uts[s][:], P[:])
                else:
                    nc.vector.tensor_copy(uts[s][:], P[:])
            return uts

        def emit_p2(im, it, uts):
            A_t = A_s if it < N_ITER - 1 else A_p
            final = it == N_ITER - 1
            if not final:
                # pass 2 s-outer so the matmuls gate on ut[s] incrementally
                DPs = [psum() for _ in range(NT)]
                for s in range(NT - 1):
                    lo, hi = _win(s)
                    for tp in range(NT):
                        nc.tensor.matmul(
                            DPs[tp][:, lo:hi], uts[s][:, 128 * tp:128 * tp + 128],
                            A_t[:, s, lo:hi],
                            start=(s == 0), stop=False, skip_group_check=True)
                lo, hi = _win(NT - 1)
                for tp in range(NT):
                    # last band window, then immediately d' = 2*blur(s) - M for
                    # this bank (A_t carries sqrt(2) per pass) so sigmoid(tp)
                    # can start 3 matmul-slots earlier
                    nc.tensor.matmul(
                        DPs[tp][:, lo:hi], uts[NT - 1][:, 128 * tp:128 * tp + 128],
                        A_t[:, NT - 1, lo:hi],
                        start=False, stop=False, skip_group_check=True)
                    nc.tensor.matmul(
                        DPs[tp][:], negI[:], M[im][tp][:],
                        start=False, stop=True, skip_group_check=True)
                dp[im] = DPs
                sig[im] = emit_sig(im, it + 1)
            else:
                # final pass 2 bank-sequential: tile tp's outputs compute and
                # DMA out while tile tp+1 is still matmuling
                for tp in range(NT):
                    P = psum()
                    for s in range(NT):
                        lo, hi = _win(s)
                        nc.tensor.matmul(
                            P[:, lo:hi], uts[s][:, 128 * tp:128 * tp + 128],
                            A_t[:, s, lo:hi],
                            start=(s == 0), stop=(s == NT - 1),
                            skip_group_check=True)
                    o1 = spool.tile([128, W], FP16, tag=f"o1_{im}_{tp}", name=f"o1_{im}_{tp}")
                    o0 = spool.tile([128, W], FP16, tag=f"o0_{im}_{tp}", name=f"o0_{im}_{tp}")
                    # out1 = blur + u1 (DVE, reads PSUM);
                    # out0 = (2*u1 + M) - out1 (Pool, SBUF-only, overlaps DVE)
                    nc.vector.tensor_add(o1[:], P[:], u1[im][tp][:])
                    nc.gpsimd.tensor_sub(o0[:], P2[im][tp][:], o1[:])
                    rows = slice(128 * tp, 128 * tp + 128)
                    nc.sync.dma_start(y_d[im, 1, rows, :], o1[:])
                    nc.gpsimd.dma_start(y_d[im, 0, rows, :], o0[:])

        def emit_iter(im, it):
            emit_p2(im, it, emit_p1(im, it))

        # --- global schedule: interleave the two images so each image's
        #     sigmoid chain hides under the other's matmuls; the tail is
        #     fine-split so the last chains get partial cover too ---
        A, B = 0, 1
        for b in range(NT):
            conv_chunk(A, b, 0)
            conv_chunk(A, b, 1)
        sig[A] = emit_sig(A, 0)
        emit_iter(A, 0)                      # emits sigA1 at its tail
        for b in range(NT):
            conv_chunk(B, b, 0)
            conv_chunk(B, b, 1)
        sig[B] = emit_sig(B, 0)
        emit_iter(A, 1)
        emit_iter(B, 0)
        emit_iter(A, 2)
        emit_iter(B, 1)
        emit_iter(A, 3)
        emit_iter(B, 2)
        utsA4 = emit_p1(A, 4)
        utsB3 = emit_p1(B, 3)
        emit_p2(B, 3, utsB3)                 # emits sigB4
        emit_p2(A, 4, utsA4)                 # A outputs
        emit_iter(B, 4)                      # B outputs


_CACHE = {}


def _get_compiled():
    key = 0
    if key in _CACHE:
        return _CACHE[key]
    nc = bacc.Bacc(
        "TRN2",
        target_bir_lowering=False,
        debug=False,
        enable_asserts=False,
        num_devices=N_CORES,
    )
    with tile.TileContext(nc) as tc:
        _build(nc, tc, FP16, FP16)
    nc.compile()
    _CACHE[key] = nc
    return nc


def host_constants(conv_w, conv_b):
    """All weight-derived device constants, as numpy arrays."""
    w = np.asarray(conv_w, np.float32)
    b = np.asarray(conv_b, np.float32)
    sets = [w[1] + 0.0, w[1] - w[0]]  # u1-plane, du-plane (3,3,3) each

    bands = np.zeros((128, 18, 128), np.float32)
    r = np.arange(128)
    for set_i, ws in enumerate(sets):
        for c in range(3):
            for kx in range(3):
                Band = np.zeros((128, 128), np.float32)
                for ky in range(3):
                    m = r - (ky - 1)
                    ok = (m >= 0) & (m < 128)
                    Band[r[ok], m[ok]] = ws[c, ky, kx]
                bands[:, set_i * 9 + c * 3 + kx, :] = Band

    wf = np.zeros((35, 6, 128), np.float32)
    for set_i, ws in enumerate(sets):
        for kx in range(3):
            WF = np.zeros((35, 128), np.float32)
            for c in range(3):
                WF[0 + c, 0] = ws[c, 0, kx]      # r=0 rows: x row 128b-1, ky=0
                WF[32 + c, 127] = ws[c, 2, kx]   # r=1 rows: x row 128b+128, ky=2
            wf[:, set_i * 3 + kx, :] = WF

    def tile4(A):
        return np.ascontiguousarray(A.reshape(NT, 128, H).transpose(1, 0, 2))

    A_s = tile4(_make_A(np.sqrt(np.float32(2.0))))
    A_p = tile4(_make_A(1.0))

    k = _gauss_k()
    v = np.convolve(np.ones(H, np.float32), k, mode="same").astype(np.float32)
    ob_full = np.outer(v, v).astype(np.float32)  # blur(ones), rank-1
    ob = np.ascontiguousarray(ob_full.reshape(NT, 128, W).transpose(1, 0, 2))

    ident = np.eye(128, dtype=np.float32)
    b1, db = np.float32(b[1]), np.float32(b[1] - b[0])
    return {
        "bands": bands.astype(np.float16),
        "wf": wf.astype(np.float16),
        "A_s": A_s.astype(np.float16),
        "A_p": A_p.astype(np.float16),
        "negI": (-ident).astype(np.float16),
        "ob": ob.astype(np.float16),
        "biases": np.tile(np.array([[b1, db]], np.float32), (128, 1)),
    }


def _install_ntff_hook_shim():
    """This container's antenv lacks axon_hooks; recreate the NTFF profile
    hook via ctypes into libaxon_pjrt.so (same ABI trn_boot.py uses).
    Only invoked for traced (profiling) runs."""
    import types
    import ctypes
    import contextlib

    try:
        from antenv.axon_hooks import get_axon_ntff_profile_hook  # noqa: F401
        return
    except ImportError:
        pass

    hook = None
    so_path = "/opt/axon/libaxon_pjrt.so"
    if os.path.exists(so_path):
        lib = ctypes.CDLL(so_path)
        if hasattr(lib, "axon_start_nrt_profile"):
            lib.axon_start_nrt_profile.argtypes = [
                ctypes.POINTER(ctypes.c_int64), ctypes.c_size_t,
            ]
            lib.axon_start_nrt_profile.restype = ctypes.c_int64
            lib.axon_stop_nrt_profile.argtypes = [ctypes.c_char_p]
            lib.axon_stop_nrt_profile.restype = ctypes.c_int64

            @contextlib.contextmanager
            def _hook(output_dir, device_ids):
                import jax

                jax.devices()
                if device_ids:
                    ids = (ctypes.c_int64 * len(device_ids))(*device_ids)
                    rc = lib.axon_start_nrt_profile(ids, len(device_ids))
                else:
                    rc = lib.axon_start_nrt_profile(None, 0)
                if rc != 0:
                    raise RuntimeError(f"axon_start_nrt_profile rc={rc}")
                try:
                    yield
                finally:
                    n = lib.axon_stop_nrt_profile(str(output_dir).encode())
                    print(f"profile: {n} file(s) written to {output_dir}", file=sys.stderr)

            hook = _hook

    import antenv

    mod = types.ModuleType("antenv.axon_hooks")
    mod.get_axon_ntff_profile_hook = lambda: hook
    mod.set_axon_ntff_profile_hook = lambda h: None
    sys.modules["antenv.axon_hooks"] = mod
    antenv.axon_hooks = mod


def kernel(x, conv_w, conv_b, _trace=False, _return_results=False):
    if _trace:
        _install_ntff_hook_shim()
    x = np.ascontiguousarray(np.asarray(x, np.float32).astype(np.float16))
    consts = host_constants(conv_w, conv_b)

    nc = _get_compiled()
    in_maps = []
    for core in range(N_CORES):
        m = {"x": np.ascontiguousarray(x[IMGS_PER_CORE * core:IMGS_PER_CORE * (core + 1)])}
        m.update(consts)
        in_maps.append(m)

    res = run_bass_kernel_spmd(nc, in_maps, core_ids=list(range(N_CORES)), trace=_trace)
    out = np.concatenate([res.results[c]["y"] for c in range(N_CORES)], axis=0).astype(np.float32)
    if _return_results:
        return out, res
    return out


if __name__ == "__main__":
    rng = np.random.default_rng(0)
    x = rng.standard_normal((16, 3, H, W), dtype=np.float32)
    w = (rng.standard_normal((2, 3, 3, 3)) * 0.1).astype(np.float32)
    b = np.zeros(2, np.float32)
    y = kernel(x=x, conv_w=w, conv_b=b)
    print("out", y.shape, y.dtype)


# revision 36
# speedup vs baseline: 1.0594x; 1.0594x over previous
"""Trainium2 Bass kernel for CRFExtensionModule (conv3x3 backbone + 5 mean-field
CRF iterations with separable Gaussian blur).

Strategy (per NeuronCore, 2 images A/B of the 16-image batch):
  - C=2 softmax collapses to a sigmoid of d = logit1 - logit0, and
    blur(q0) = blur(ones) - blur(q1), so the whole CRF loop is a single-plane
    recurrence:  d' = (du - ob) + 2*blur(sigmoid(d)),  ob = blur(ones).
  - conv3x3 computes only the planes u1 and du = u1 - u0 via banded matmuls on
    the TensorEngine (band stationary, x moving), with tiny K=35 "fix" matmuls
    for the 2 rows per 128-row tile boundary that the aligned K window misses.
  - blur = two *transposing* banded matmul passes on the TensorEngine:
    pass1: UT[w,h'] = sum_h s[h,w] A[h,h']   (lhsT = s block, rhs = A band)
    pass2: V[h,w'] = sum_w UT[w,h] A[w,w']   (lhsT = UT block, rhs = A band)
  - M = ob - du is injected into the pass-2 PSUM with K=128 negI matmuls, so
    sigmoid reads PSUM directly (d' = 2*blur(s) - M lives in the bank).
  - Scheduling: the two images' CRF iterations are interleaved on the PE
    (A0 | convB | A1 | B0 | A2 | B1 | ...), so image X's sigmoid chain runs
    under image Y's matmuls and the PE almost never stalls.
  - Consts stream on the (otherwise idle) Sync HWDGE ring; x streams on the
    SWDGE cast ring in (channel-interleaved, row-tile) order so the first conv
    matmul can start ~5us in.  Outputs are written fp16 (tolerance is 2e-2;
    fp16 adds ~5e-4) and all output DMAs issue from the Sync ring.
"""

import os
import sys
from contextlib import ExitStack

sys.path.insert(0, "/opt/trn_rl_repo")

import numpy as np
import ml_dtypes

import concourse.bass as bass
import concourse.bacc as bacc
import concourse.tile as tile
import concourse.mybir as mybir
from concourse.bass_utils import run_bass_kernel_spmd

F32 = mybir.dt.float32
BF16 = mybir.dt.bfloat16
FP16 = mybir.dt.float16

N_CORES = 8
IMGS_PER_CORE = 2
H = W = 512
NT = 4  # 128-row tiles per image plane
N_ITER = 5
FILT = 11


def _gauss_k():
    d = np.arange(FILT, dtype=np.float32) - np.float32((FILT - 1) / 2.0)
    k = np.exp(-(d ** 2) / np.float32(2.0)).astype(np.float32)
    return (k / k.sum()).astype(np.float32)


def _make_A(scale):
    """A[h, h'] = k[h-h'+5] for |h-h'| <= 5 (zero-padded 'SAME' 1D blur)."""
    k = (_gauss_k() * np.float32(scale)).astype(np.float32)
    A = np.zeros((H, H), np.float32)
    hp = np.arange(H)
    for j in range(FILT):
        h = hp + (j - 5)
        m = (h >= 0) & (h < H)
        A[h[m], hp[m]] = k[j]
    return A


def _win(t):
    """h' window that rows [128t, 128t+128) of A touch."""
    return max(0, 128 * t - 5), min(H, 128 * t + 133)


def _np_dt(dt):
    if dt == BF16:
        return ml_dtypes.bfloat16
    if dt == FP16:
        return np.float16
    return np.float32


# ---------------------------------------------------------------------------
# kernel body (traced once; shared SPMD program for all 8 cores)
# ---------------------------------------------------------------------------


def _build(nc, tc, conv_dt, blur_dt):
    x_d = nc.dram_tensor("x", [IMGS_PER_CORE, 3, H, W], FP16, kind="ExternalInput").ap()
    y_d = nc.dram_tensor("y", [IMGS_PER_CORE, 2, H, W], FP16, kind="ExternalOutput").ap()
    bands_d = nc.dram_tensor("bands", [128, 18, 128], conv_dt, kind="ExternalInput").ap()
    wf_d = nc.dram_tensor("wf", [35, 6, 128], conv_dt, kind="ExternalInput").ap()
    As_d = nc.dram_tensor("A_s", [128, NT, H], blur_dt, kind="ExternalInput").ap()
    Ap_d = nc.dram_tensor("A_p", [128, NT, H], blur_dt, kind="ExternalInput").ap()
    negI_d = nc.dram_tensor("negI", [128, 128], FP16, kind="ExternalInput").ap()
    ob_d = nc.dram_tensor("ob", [128, NT, W], FP16, kind="ExternalInput").ap()
    biases_d = nc.dram_tensor("biases", [128, 2], F32, kind="ExternalInput").ap()

    ALU = mybir.AluOpType
    AF = mybir.ActivationFunctionType

    with ExitStack() as ctx:
        cpool = ctx.enter_context(tc.tile_pool(name="consts", bufs=1))
        spool = ctx.enter_context(tc.tile_pool(name="sbuf", bufs=1))
        xpool = ctx.enter_context(tc.tile_pool(name="xin", bufs=1))
        ppool = ctx.enter_context(
            tc.tile_pool(name="psum", bufs=8, space=bass.MemorySpace.PSUM))

        def psum():
            return ppool.tile([128, 512], F32, tag="ps", name="ps")

        # --- constants into SBUF on the Sync HWDGE ring, in order of first
        #     need (Scalar's ring is blocked by ACT table loads; Sync is
        #     otherwise idle).  x goes on the SWDGE ring. ---
        bands = cpool.tile([128, 18, 128], conv_dt, tag="bands", name="bands")
        nc.sync.dma_start(bands[:], bands_d)
        biases = cpool.tile([128, 2], F32, tag="biases", name="biases")
        nc.sync.dma_start(biases[:], biases_d)
        wf = cpool.tile([35, 6, 128], conv_dt, tag="wf", name="wf")
        nc.sync.dma_start(wf[:], wf_d)
        A_s = cpool.tile([128, NT, H], blur_dt, tag="A_s", name="A_s")
        nc.sync.dma_start(A_s[:], As_d)
        ob = cpool.tile([128, NT, W], FP16, tag="ob", name="ob")
        nc.sync.dma_start(ob[:], ob_d)
        negI = cpool.tile([128, 128], FP16, tag="negI", name="negI")
        nc.sync.dma_start(negI[:], negI_d)
        A_p = cpool.tile([128, NT, H], blur_dt, tag="A_p", name="A_p")
        nc.sync.dma_start(A_p[:], Ap_d)

        # --- per-image state ---
        xt, xbt, u1, du, M, P2, s_sb, ut, dp = {}, {}, {}, {}, {}, {}, {}, {}, {}
        for im in range(IMGS_PER_CORE):
            xt[im] = xpool.tile([128, 3, NT, W], conv_dt, tag=f"xt{im}", name=f"xt{im}")
            xbt[im] = [xpool.tile([35, W], conv_dt, tag=f"xb{im}{b}", name=f"xb{im}{b}")
                       for b in range(NT)]
            u1[im] = [spool.tile([128, W], F32, tag=f"u1_{im}_{b}", name=f"u1_{im}_{b}") for b in range(NT)]
            du[im] = [spool.tile([128, W], F32, tag=f"du_{im}_{b}", name=f"du_{im}_{b}") for b in range(NT)]
            M[im] = [spool.tile([128, W], FP16, tag=f"M_{im}_{b}", name=f"M_{im}_{b}") for b in range(NT)]
            P2[im] = [spool.tile([128, W], F32, tag=f"P2_{im}_{b}", name=f"P2_{im}_{b}") for b in range(NT)]
            dp[im] = None

        def s_tiles(im):
            return [spool.tile([128, W], blur_dt, tag=f"s_{im}_{t}", name=f"s_{im}_{t}") for t in range(NT)]

        def ut_tiles(im):
            return [spool.tile([128, H], blur_dt, tag=f"ut_{im}_{s}", name=f"ut_{im}_{s}") for s in range(NT)]

        # --- x DMAs: memsets first (vector), then fp16 loads in (tile,
        #     channel) order so conv tile b is fed just in time ---
        for im in range(IMGS_PER_CORE):
            for b in range(NT):
                nc.vector.memset(xbt[im][b][:], 0.0)

        def x_tile(im, b, ring):
            for c in range(3):
                ring.dma_start(
                    xt[im][:, c, b, :], x_d[im, c, 128 * b:128 * b + 128, :])
            if b > 0:
                ring.dma_start(xbt[im][b][0:3, :], x_d[im, :, 128 * b - 1, :])
            if b < NT - 1:
                ring.dma_start(xbt[im][b][32:35, :], x_d[im, :, 128 * b + 128, :])

        for b in range(NT):
            x_tile(0, b, nc.gpsimd)
        # image B: per-channel whole-plane issues (fewer descriptors on the
        # SWDGE queue); its boundary strips ride the Sync ring
        for c in range(3):
            nc.gpsimd.dma_start(
                xt[1][:, c, :, :],
                x_d[1, c].rearrange("(b p) w -> p b w", p=128))
        for b in range(NT):
            if b > 0:
                nc.sync.dma_start(xbt[1][b][0:3, :], x_d[1, :, 128 * b - 1, :])
            if b < NT - 1:
                nc.sync.dma_start(xbt[1][b][32:35, :], x_d[1, :, 128 * b + 128, :])

        # --- emission helpers (program order == per-engine issue order) ---

        def conv_chunk(im, b, set_i):
            P = psum()
            n_mm = 0
            for c in range(3):
                for kx in (1, 0, 2):  # center first: full-bank start=True
                    sl, ol = (0, 1) if kx == 0 else (1, 0) if kx == 2 else (0, 0)
                    n = W - (1 if kx != 1 else 0)
                    nc.tensor.matmul(
                        P[:, ol:ol + n],
                        bands[:, set_i * 9 + c * 3 + kx, :],
                        xt[im][:, c, b, sl:sl + n],
                        start=(n_mm == 0), stop=False, skip_group_check=True)
                    n_mm += 1
            for kx in (1, 0, 2):
                sl, ol = (0, 1) if kx == 0 else (1, 0) if kx == 2 else (0, 0)
                n = W - (1 if kx != 1 else 0)
                nc.tensor.matmul(
                    P[:, ol:ol + n], wf[:, set_i * 3 + kx, :],
                    xbt[im][b][:, sl:sl + n],
                    start=False, stop=(kx == 2), skip_group_check=True)
            if set_i == 0:
                # u1 = conv1 + b1   (ACT)
                nc.scalar.activation(
                    u1[im][b][:], P[:], AF.Identity, bias=biases[:, 0:1], scale=1.0)
            else:
                # du = (conv1-conv0) + (b1-b0)   (DVE)
                nc.vector.tensor_scalar(
                    du[im][b][:], P[:], biases[:, 1:2], None, ALU.add)
                # M = ob - du (fp16) ; P2 = u1 + M   (Pool: SBUF-only operands,
                # and Pool is otherwise just issuing x DMAs)
                nc.gpsimd.tensor_sub(M[im][b][:], ob[:, b, :], du[im][b][:])
                nc.gpsimd.tensor_add(P2[im][b][:], u1[im][b][:], M[im][b][:])

        def emit_sig(im, it):
            """s = sigmoid(d) for the next iteration's pass 1 (ACT)."""
            s_sb = s_tiles(im)
            src = du[im] if it == 0 else dp[im]
            for t in range(NT):
                nc.scalar.activation(s_sb[t][:], src[t][:], AF.Sigmoid)
            return s_sb

        sig = {}  # im -> current s tiles

        def emit_p1(im, it):
            A_t = A_s if it < N_ITER - 1 else A_p
            s_sb = sig[im]
            # pass 1 (bank-sequential: bank s completes at (s+1)/4 of the pass
            # so its SBUF copy overlaps the rest of the pass)
            uts = ut_tiles(im)
            for s in range(NT):
                P = psum()
                for t in range(NT):
                    lo, hi = _win(t)
                    nc.tensor.matmul(
                        P[:, lo:hi], s_sb[t][:, 128 * s:128 * s + 128],
                        A_t[:, t, lo:hi],
                        start=(t == 0), stop=(t == NT - 1), skip_group_check=True)
                # copy engines: s=0 first-needed by pass 2, s=3 last
                # (Pool cannot touch PSUM, so DVE takes 3 of 4)
                if s == 1:
                    nc.scalar.copy(uts[s][:], P[:])
                else:
                    nc.vector.tensor_copy(uts[s][:], P[:])
            return uts

        def emit_p2(im, it, uts):
            A_t = A_s if it < N_ITER - 1 else A_p
            final = it == N_ITER - 1
            if not final:
                # pass 2 s-outer so the matmuls gate on ut[s] incrementally
                DPs = [psum() for _ in range(NT)]
                for s in range(NT - 1):
                    lo, hi = _win(s)
                    for tp in range(NT):
                        nc.tensor.matmul(
                            DPs[tp][:, lo:hi], uts[s][:, 128 * tp:128 * tp + 128],
                            A_t[:, s, lo:hi],
                            start=(s == 0), stop=False, skip_group_check=True)
                lo, hi = _win(NT - 1)
                for tp in range(NT):
                    # last band window, then immediately d' = 2*blur(s) - M for
                    # this bank (A_t carries sqrt(2) per pass) so sigmoid(tp)
                    # can start 3 matmul-slots earlier
                    nc.tensor.matmul(
                        DPs[tp][:, lo:hi], uts[NT - 1][:, 128 * tp:128 * tp + 128],
                        A_t[:, NT - 1, lo:hi],
                        start=False, stop=False, skip_group_check=True)
                    nc.tensor.matmul(
                        DPs[tp][:], negI[:], M[im][tp][:],
                        start=False, stop=True, skip_group_check=True)
                dp[im] = DPs
                sig[im] = emit_sig(im, it + 1)
            else:
                # final pass 2 bank-sequential: tile tp's outputs compute and
                # DMA out while tile tp+1 is still matmuling
                for tp in range(NT):
                    P = psum()
                    for s in range(NT):
                        lo, hi = _win(s)
                        nc.tensor.matmul(
                            P[:, lo:hi], uts[s][:, 128 * tp:128 * tp + 128],
                            A_t[:, s, lo:hi],
                            start=(s == 0), stop=(s == NT - 1),
                            skip_group_check=True)
                    o1 = spool.tile([128, W], FP16, tag=f"o1_{im}_{tp}", name=f"o1_{im}_{tp}")
                    o0 = spool.tile([128, W], FP16, tag=f"o0_{im}_{tp}", name=f"o0_{im}_{tp}")
                    # out1 = blur + u1 ; out0 = P2 - blur  (PSUM reads: DVE)
                    nc.vector.tensor_add(o1[:], P[:], u1[im][tp][:])
                    nc.vector.scalar_tensor_tensor(
                        o0[:], P[:], -1.0, P2[im][tp][:], ALU.mult, ALU.add)
                    rows = slice(128 * tp, 128 * tp + 128)
                    nc.sync.dma_start(y_d[im, 1, rows, :], o1[:])
                    nc.gpsimd.dma_start(y_d[im, 0, rows, :], o0[:])

        def emit_iter(im, it):
            emit_p2(im, it, emit_p1(im, it))

        # --- global schedule: interleave the two images so each image's
        #     sigmoid chain hides under the other's matmuls; the tail is
        #     fine-split so the last chains get partial cover too ---
        A, B = 0, 1
        for b in range(NT):
            conv_chunk(A, b, 0)
            conv_chunk(A, b, 1)
        sig[A] = emit_sig(A, 0)
        emit_iter(A, 0)                      # emits sigA1 at its tail
        for b in range(NT):
            conv_chunk(B, b, 0)
            conv_chunk(B, b, 1)
        sig[B] = emit_sig(B, 0)
        emit_iter(A, 1)
        emit_iter(B, 0)
        emit_iter(A, 2)
        emit_iter(B, 1)
        emit_iter(A, 3)
        emit_iter(B, 2)
        utsA4 = emit_p1(A, 4)
        utsB3 = emit_p1(B, 3)
        emit_p2(B, 3, utsB3)                 # emits sigB4
        emit_p2(A, 4, utsA4)                 # A outputs
        emit_iter(B, 4)                      # B outputs


_CACHE = {}


def _get_compiled():
    key = 0
    if key in _CACHE:
        return _CACHE[key]
    nc = bacc.Bacc(
        "TRN2",
        target_bir_lowering=False,
        debug=False,
        enable_asserts=False,
        num_devices=N_CORES,
    )
    with tile.TileContext(nc) as tc:
        _build(nc, tc, FP16, FP16)
    nc.compile()
    _CACHE[key] = nc
    return nc


def host_constants(conv_w, conv_b):
    """All weight-derived device constants, as numpy arrays."""
    w = np.asarray(conv_w, np.float32)
    b = np.asarray(conv_b, np.float32)
    sets = [w[1] + 0.0, w[1] - w[0]]  # u1-plane, du-plane (3,3,3) each

    bands = np.zeros((128, 18, 128), np.float32)
    r = np.arange(128)
    for set_i, ws in enumerate(sets):
        for c in range(3):
            for kx in range(3):
                Band = np.zeros((128, 128), np.float32)
                for ky in range(3):
                    m = r - (ky - 1)
                    ok = (m >= 0) & (m < 128)
                    Band[r[ok], m[ok]] = ws[c, ky, kx]
                bands[:, set_i * 9 + c * 3 + kx, :] = Band

    wf = np.zeros((35, 6, 128), np.float32)
    for set_i, ws in enumerate(sets):
        for kx in range(3):
            WF = np.zeros((35, 128), np.float32)
            for c in range(3):
                WF[0 + c, 0] = ws[c, 0, kx]      # r=0 rows: x row 128b-1, ky=0
                WF[32 + c, 127] = ws[c, 2, kx]   # r=1 rows: x row 128b+128, ky=2
            wf[:, set_i * 3 + kx, :] = WF

    def tile4(A):
        return np.ascontiguousarray(A.reshape(NT, 128, H).transpose(1, 0, 2))

    A_s = tile4(_make_A(np.sqrt(np.float32(2.0))))
    A_p = tile4(_make_A(1.0))

    k = _gauss_k()
    v = np.convolve(np.ones(H, np.float32), k, mode="same").astype(np.float32)
    ob_full = np.outer(v, v).astype(np.float32)  # blur(ones), rank-1
    ob = np.ascontiguousarray(ob_full.reshape(NT, 128, W).transpose(1, 0, 2))

    ident = np.eye(128, dtype=np.float32)
    b1, db = np.float32(b[1]), np.float32(b[1] - b[0])
    return {
        "bands": bands.astype(np.float16),
        "wf": wf.astype(np.float16),
        "A_s": A_s.astype(np.float16),
        "A_p": A_p.astype(np.float16),
        "negI": (-ident).astype(np.float16),
        "ob": ob.astype(np.float16),
        "biases": np.tile(np.array([[b1, db]], np.float32), (128, 1)),
    }


def _install_ntff_hook_shim():
    """This container's antenv lacks axon_hooks; recreate the NTFF profile
    hook via ctypes into libaxon_pjrt.so (same ABI trn_boot.py uses).
    Only invoked for traced (profiling) runs."""
    import types
    import ctypes
    import contextlib

    try:
        from antenv.axon_hooks import get_axon_ntff_profile_hook  # noqa: F401
        return
    except ImportError:
        pass

    hook = None
    so_path = "/opt/axon/libaxon_pjrt.so"
    if os.path.exists(so_path):
        lib = ctypes.CDLL(so_path)
        if hasattr(lib, "axon_start_nrt_profile"):
            lib.axon_start_nrt_profile.argtypes = [
                ctypes.POINTER(ctypes.c_int64), ctypes.c_size_t,
            ]
            lib.axon_start_nrt_profile.restype = ctypes.c_int64
            lib.axon_stop_nrt_profile.argtypes = [ctypes.c_char_p]
            lib.axon_stop_nrt_profile.restype = ctypes.c_int64

            @contextlib.contextmanager
            def _hook(output_dir, device_ids):
                import jax

                jax.devices()
                if device_ids:
                    ids = (ctypes.c_int64 * len(device_ids))(*device_ids)
                    rc = lib.axon_start_nrt_profile(ids, len(device_ids))
                else:
                    rc = lib.axon_start_nrt_profile(None, 0)
                if rc != 0:
                    raise RuntimeError(f"axon_start_nrt_profile rc={rc}")
                try:
                    yield
                finally:
                    n = lib.axon_stop_nrt_profile(str(output_dir).encode())
                    print(f"profile: {n} file(s) written to {output_dir}", file=sys.stderr)

            hook = _hook

    import antenv

    mod = types.ModuleType("antenv.axon_hooks")
    mod.get_axon_ntff_profile_hook = lambda: hook
    mod.set_axon_ntff_profile_hook = lambda h: None
    sys.modules["antenv.axon_hooks"] = mod
    antenv.axon_hooks = mod


def kernel(x, conv_w, conv_b, _trace=False, _return_results=False):
    if _trace:
        _install_ntff_hook_shim()
    x = np.ascontiguousarray(np.asarray(x, np.float32).astype(np.float16))
    consts = host_constants(conv_w, conv_b)

    nc = _get_compiled()
    in_maps = []
    for core in range(N_CORES):
        m = {"x": np.ascontiguousarray(x[IMGS_PER_CORE * core:IMGS_PER_CORE * (core + 1)])}
        m.update(consts)
        in_maps.append(m)

    res = run_bass_kernel_spmd(nc, in_maps, core_ids=list(range(N_CORES)), trace=_trace)
    out = np.concatenate([res.results[c]["y"] for c in range(N_CORES)], axis=0).astype(np.float32)
    if _return_results:
        return out, res
    return out


if __name__ == "__main__":
    rng = np.random.default_rng(0)
    x = rng.standard_normal((16, 3, H, W), dtype=np.float32)
    w = (rng.standard_normal((2, 3, 3, 3)) * 0.1).astype(np.float32)
    b = np.zeros(2, np.float32)
    y = kernel(x=x, conv_w=w, conv_b=b)
    print("out", y.shape, y.dtype)
